# revision 2
# baseline (speedup 1.0000x reference)
"""MlpAttentionLayer Trainium2 kernel, v3.

Math (reference):
  cat = [x, x-q, q]; h = BN1(cat); p = relu(h @ W1)
  g = BN2(p); w = sigmoid(g @ W2); out = sum_t x * w

Folding (host): pre = x @ Wx + Qp[b]; logits = relu(pre) @ W2p + c2;
out[b] = sum_t x[b,t] * sigmoid(logits[b,t]).

Device layout (per core, 256 b):
  x host-padded to 256 tokens (zeros) and loaded as token-pair tiles
  [128 parts, b, 2, 128] fp32 - 1KB contiguous HBM chunks on all 128
  partitions (all 16 SDMA engines engaged). No bf16 cast pass: PE
  transposes the fp32 pairs directly (transpose-mode is full rate for
  fp32 on trn2); the PSUM->SBUF drain does the bf16 cast (DVE). Main
  matmul per 2-b: pre = Wx^T @ xT (N=400, bf16). relu+Qp via ACT bias
  per b (1 pass, PSUM->SBUF bf16). Logits per (b,j): N=1 matmul
  lhsT=h1-chunk (M=100) rhs=W2p-col -> PSUM column pack = wT layout
  directly. Sigmoid per 32-b chunk (ACT, bias c2, fp32 out). Final per
  (b,j): fout[:, b] += x32-chunk @ w-chunk in fp32 (N=1, PSUM accum).
  Epilogue transposes fout [d,b] -> [b,d].
"""

import sys

sys.path.insert(0, "/opt/trn_rl_repo")

import numpy as np
import ml_dtypes

BN_EPS = 1e-3
B, T, D = 2048, 200, 128
N_CORES = 8
BSH = B // N_CORES          # 256 batch elements per core
G = 4                       # batch elements per pipeline group
NGRP = BSH // G             # 64 groups
P = 100                     # real token-pair count (200 = 100 pairs)
TPAD = 256                  # host-padded token count (128 pairs)
PP = TPAD // 2              # 128 partitions in the load tiles
CH = 112                    # padded xT/h1 chunk stride (cols per j), %16
WCHUNK = 32                 # b's per sigmoid batch
GPW = WCHUNK // G           # groups per sigmoid batch (8)

BF16 = ml_dtypes.bfloat16
DEBUG = False


def _build_bass():
    from concourse import bacc, mybir
    from concourse.tile import TileContext
    from concourse.masks import make_identity

    fp32 = mybir.dt.float32
    bf16 = mybir.dt.bfloat16
    AF = mybir.ActivationFunctionType

    nc = bacc.Bacc()
    x_d = nc.dram_tensor("x", (BSH, TPAD, D), fp32, kind="ExternalInput")
    qpt_d = nc.dram_tensor("qpt", (D, BSH), fp32, kind="ExternalInput")
    wx_d = nc.dram_tensor("wx", (D, D), bf16, kind="ExternalInput")
    w2c_d = nc.dram_tensor("w2c", (D, 1), bf16, kind="ExternalInput")
    c2_d = nc.dram_tensor("c2", (1, 1), fp32, kind="ExternalInput")
    out_d = nc.dram_tensor("out", (BSH, D), fp32, kind="ExternalOutput")

    with TileContext(nc) as tc:
        with (
            tc.tile_pool(name="const", bufs=1) as cpool,
            tc.tile_pool(name="xin", bufs=4) as xpool,
            tc.tile_pool(name="x16", bufs=2 * GPW + 4) as x16pool,
            tc.tile_pool(name="xt", bufs=3) as xtpool,
            tc.tile_pool(name="h1", bufs=3) as h1pool,
            tc.tile_pool(name="wt", bufs=2) as wtpool,
            tc.tile_pool(name="fin", bufs=1) as finpool,
            tc.tile_pool(name="ps_xt", bufs=2, space="PSUM") as xtp_pool,
            tc.tile_pool(name="ps_pre", bufs=3, space="PSUM") as pre_pool,
            tc.tile_pool(name="ps_w", bufs=2, space="PSUM") as wps_pool,
            tc.tile_pool(name="ps_out", bufs=1, space="PSUM") as fout_pool,
        ):
            ident16 = cpool.tile([128, 128], bf16)
            make_identity(nc, ident16)
            ident32 = cpool.tile([128, 128], fp32)
            make_identity(nc, ident32)
            wx_sb = cpool.tile([D, D], bf16)
            nc.sync.dma_start(wx_sb, wx_d[:, :])
            w2c_sb = cpool.tile([D, 1], bf16)
            nc.sync.dma_start(w2c_sb, w2c_d[:, :])
            c2_sb = cpool.tile([128, 1], fp32)
            nc.sync.dma_start(c2_sb, c2_d[0, 0:1].broadcast_to((128, 1)))
            qpt_sb = cpool.tile([D, BSH], fp32)
            nc.sync.dma_start(qpt_sb, qpt_d[:, :])

            fout = fout_pool.tile([128, BSH], fp32)

            x16_tiles = [None] * NGRP
            wt_tiles = [None] * (NGRP // GPW)

            def do_final(ci):
                wt = wt_tiles[ci]
                for gg in range(GPW):
                    gsrc = ci * GPW + gg
                    xs = x16_tiles[gsrc]
                    for g in range(G):
                        bc = gsrc * G + g
                        bl = gg * G + g
                        for j in range(2):
                            nc.tensor.matmul(
                                fout[:, bc : bc + 1],
                                xs[0:P, g, j, :],
                                wt[:, j, bl : bl + 1],
                                start=(j == 0),
                                stop=(j == 1),
                            )
                    x16_tiles[gsrc] = None

            for gi in range(NGRP):
                b0 = gi * G
                ci = gi // GPW          # 32-b chunk index
                gl = gi % GPW           # group-in-chunk
                if gl == 0:
                    wps = wps_pool.tile([128, 2, WCHUNK], fp32, tag="wps")

                # ---- load fp32 pair tiles (1KB HBM chunks, 128 partitions)
                x32 = xpool.tile([PP, G, 2 * D], fp32, tag="x32")
                nc.sync.dma_start(
                    x32,
                    x_d[b0 : b0 + G, :, :].rearrange(
                        "b (p two) d -> p b (two d)", two=2
                    ),
                )
                # ---- cast to bf16 (DVE)
                x16 = x16pool.tile([PP, G, 2, D], bf16, tag="x16")
                nc.vector.tensor_copy(
                    x16.rearrange("p g two d -> p (g two d)"),
                    x32.rearrange("p g twod -> p (g twod)"),
                )
                x16_tiles[gi] = x16

                # ---- xT via bf16 PE transposes; int32-view PSUM drain
                xt = xtpool.tile([128, G, 2, CH], bf16, tag="xt")
                xtp = xtp_pool.tile([128, G, 2, CH], bf16, tag="xtp")
                for g in range(G):
                    for j in range(2):
                        nc.tensor.transpose(
                            xtp[:, g, j, 0:P],
                            x16[0:P, g, j, :],
                            ident16[0:P, 0:P],
                        )
                int32 = mybir.dt.int32
                nc.vector.tensor_copy(
                    xt.bitcast(int32).rearrange("p g two c -> p (g two c)"),
                    xtp.bitcast(int32).rearrange("p g two c -> p (g two c)"),
                )

                # ---- main matmul per (b, j): pre = Wx^T @ xT (N=100)
                h1 = h1pool.tile([128, G, 2, D], bf16, tag="h1")
                for g in range(G):
                    pre = pre_pool.tile([128, 2, 2 * CH], fp32, tag="pre")
                    for j in range(2):
                        nc.tensor.matmul(
                            pre[:, j, 0:P],
                            wx_sb,
                            xt[:, g, j, 0:P],
                            start=True,
                            stop=True,
                        )
                    bc = b0 + g
                    # ---- relu + Qp bias, PSUM -> SBUF bf16 (1 pass)
                    nc.scalar.activation(
                        h1[:, g, :, 0:P],
                        pre[:, :, 0:P],
                        AF.Relu,
                        bias=qpt_sb[:, bc : bc + 1],
                    )
                    # ---- logits: N=1 matmuls -> wT column pack
                    for j in range(2):
                        nc.tensor.matmul(
                            wps[:, j, gl * G + g : gl * G + g + 1],
                            h1[:, g, j, :],
                            w2c_sb,
                            start=True,
                            stop=True,
                        )

                # ---- sigmoid per 32-b chunk -> wT fp32; then finals
                if gl == GPW - 1:
                    wt = wtpool.tile([P, 2, WCHUNK], bf16, tag="wt")
                    nc.scalar.activation(
                        wt, wps[0:P], AF.Sigmoid, bias=c2_sb[0:P, 0:1]
                    )
                    wt_tiles[ci] = wt
                    do_final(ci)

            # ---- epilogue: transpose [d, b] -> [b, d] and store
            osb = finpool.tile([128, BSH], fp32)
            nc.scalar.activation(osb, fout, AF.Copy)
            obt = finpool.tile([128, BSH], fp32)
            for half in range(2):
                ot = pre_pool.tile([128, 2, 2 * CH], fp32, tag="pre")
                otv = ot.rearrange("p a c -> p (a c)")
                nc.tensor.transpose(
                    otv[:, 0:128], osb[:, half * 128 : half * 128 + 128], ident32
                )
                nc.scalar.activation(
                    obt[:, half * 128 : half * 128 + 128], otv[:, 0:128], AF.Copy
                )
                nc.sync.dma_start(
                    out_d[half * 128 : half * 128 + 128, :],
                    obt[:, half * 128 : half * 128 + 128],
                )
    nc.finalize()
    return nc


_NC_CACHE = {}


def _get_nc():
    if "nc" not in _NC_CACHE:
        _NC_CACHE["nc"] = _build_bass()
    return _NC_CACHE["nc"]


def _host_prep(inputs, query, W1, W2, bn1_gamma, bn1_beta, bn1_mean, bn1_var,
               bn2_gamma, bn2_beta, bn2_mean, bn2_var):
    x0 = np.asarray(inputs, np.float32)
    x = np.zeros((B, TPAD, D), np.float32)
    x[:, 0:T, :] = x0
    q = np.asarray(query, np.float64)
    W1 = np.asarray(W1, np.float64)
    W2 = np.asarray(W2, np.float64)
    s1 = np.asarray(bn1_gamma, np.float64) / np.sqrt(
        np.asarray(bn1_var, np.float64) + BN_EPS
    )
    W1s = s1[:, None] * W1
    Wx = W1s[0:D] + W1s[D : 2 * D]
    Wq = W1s[2 * D : 3 * D] - W1s[D : 2 * D]
    bias0 = (np.asarray(bn1_beta, np.float64) - np.asarray(bn1_mean, np.float64) * s1) @ W1
    Qp = q @ Wq + bias0                          # [B, D]
    s2 = np.asarray(bn2_gamma, np.float64) / np.sqrt(
        np.asarray(bn2_var, np.float64) + BN_EPS
    )
    W2p = s2 * W2[:, 0]                          # [D]
    c2 = float(
        (np.asarray(bn2_beta, np.float64) - np.asarray(bn2_mean, np.float64) * s2)
        @ W2[:, 0]
    )
    wx16 = np.ascontiguousarray(Wx.astype(BF16))
    w2c16 = np.ascontiguousarray(W2p.astype(BF16)[:, None])       # [D, 1]
    qpt = np.ascontiguousarray(Qp.astype(np.float32).T)           # [D, B]
    c2a = np.full((1, 1), c2, np.float32)
    return x, qpt, wx16, w2c16, c2a


def kernel(inputs, query, W1, W2,
           bn1_gamma, bn1_beta, bn1_mean, bn1_var,
           bn2_gamma, bn2_beta, bn2_mean, bn2_var):
    from concourse.bass_utils import run_bass_kernel_spmd

    x, qpt, wx16, w2c16, c2a = _host_prep(
        inputs, query, W1, W2, bn1_gamma, bn1_beta, bn1_mean, bn1_var,
        bn2_gamma, bn2_beta, bn2_mean, bn2_var)

    nc = _get_nc()
    in_maps = []
    for c in range(N_CORES):
        in_maps.append(
            {
                "x": x[c * BSH : (c + 1) * BSH],
                "qpt": np.ascontiguousarray(qpt[:, c * BSH : (c + 1) * BSH]),
                "wx": wx16,
                "w2c": w2c16,
                "c2": c2a,
            }
        )
    res = run_bass_kernel_spmd(nc, in_maps, core_ids=list(range(N_CORES)))
    out = np.concatenate([r["out"] for r in res.results], axis=0)
    return out.astype(np.float32)


# revision 4
# speedup vs baseline: 1.0344x; 1.0344x over previous
"""MlpAttentionLayer Trainium2 kernel.

Math (reference):
  cat = [x, x-q, q]; h = BN1(cat); p = relu(h @ W1)
  g = BN2(p); w = sigmoid(g @ W2); out = sum_t x * w

Folding (host): pre = x @ Wx + Qp[b]; logits = relu(pre) @ W2p + c2;
out[b] = sum_t x[b,t] * sigmoid(logits[b,t]).

Device layout (per core, 256 b):
  x host-padded to 256 tokens (zeros) and loaded as token-pair tiles
  [128 parts, b, 2, 128] fp32 - 1KB contiguous HBM chunks on all 128
  partitions so every SDMA engine gets work (zero-pad rows contribute
  nothing to the final sum, so no masking is needed). Per group of 4 b:
  DVE cast to bf16, PE transpose per (b, j-parity): [100,128] ->
  xT [128,100] in PSUM, int32-view PSUM->SBUF drain (DVE), main matmul
  per (b,j): pre = Wx^T @ xT (N=100, bf16, Wx stationary), relu+Qp via
  one ACT pass per b (per-partition bias = QpT column, PSUM->SBUF
  bf16), logits per (b,j): N=1 matmul lhsT=h1-chunk rhs=W2p-col whose
  PSUM column packing IS the wT layout (no wT transposes), sigmoid per
  32-b chunk (ACT, bias c2, bf16), final per (b,j): fout[:, b] +=
  x16-chunk @ w-chunk (N=1, PSUM accum). Epilogue transposes
  fout [d,b] -> [b,d] and stores.
"""

import sys

sys.path.insert(0, "/opt/trn_rl_repo")

import numpy as np
import ml_dtypes

BN_EPS = 1e-3
B, T, D = 2048, 200, 128
N_CORES = 8
BSH = B // N_CORES          # 256 batch elements per core
G = 4                       # batch elements per pipeline group
NGRP = BSH // G             # 64 groups
P = 100                     # real token-pair count (200 = 100 pairs)
TPAD = 256                  # host-padded token count (128 pairs)
PP = TPAD // 2              # 128 partitions in the load tiles
CH = 112                    # padded xT/h1 chunk stride (cols per j), %16
WCHUNK = 8                  # b's per sigmoid batch
GPW = WCHUNK // G           # groups per sigmoid batch (8)

BF16 = ml_dtypes.bfloat16
DEBUG = False


def _build_bass():
    from concourse import bacc, mybir
    from concourse.tile import TileContext
    from concourse.masks import make_identity

    fp32 = mybir.dt.float32
    bf16 = mybir.dt.bfloat16
    AF = mybir.ActivationFunctionType

    nc = bacc.Bacc()
    x_d = nc.dram_tensor("x", (BSH, TPAD, D), fp32, kind="ExternalInput")
    qpt_d = nc.dram_tensor("qpt", (D, BSH), fp32, kind="ExternalInput")
    wx_d = nc.dram_tensor("wx", (D, D), bf16, kind="ExternalInput")
    w2c_d = nc.dram_tensor("w2c", (D, 1), bf16, kind="ExternalInput")
    c2_d = nc.dram_tensor("c2", (1, 1), fp32, kind="ExternalInput")
    out_d = nc.dram_tensor("out", (BSH, D), fp32, kind="ExternalOutput")

    with TileContext(nc) as tc:
        with (
            tc.tile_pool(name="const", bufs=1) as cpool,
            tc.tile_pool(name="xin", bufs=4) as xpool,
            tc.tile_pool(name="x16", bufs=2 * GPW + 4) as x16pool,
            tc.tile_pool(name="xt", bufs=3) as xtpool,
            tc.tile_pool(name="h1", bufs=3) as h1pool,
            tc.tile_pool(name="wt", bufs=2) as wtpool,
            tc.tile_pool(name="fin", bufs=1) as finpool,
            tc.tile_pool(name="ps_xt", bufs=2, space="PSUM") as xtp_pool,
            tc.tile_pool(name="ps_pre", bufs=3, space="PSUM") as pre_pool,
            tc.tile_pool(name="ps_w", bufs=2, space="PSUM") as wps_pool,
            tc.tile_pool(name="ps_out", bufs=1, space="PSUM") as fout_pool,
        ):
            ident16 = cpool.tile([128, 128], bf16)
            make_identity(nc, ident16)
            ident32 = cpool.tile([128, 128], fp32)
            make_identity(nc, ident32)
            wx_sb = cpool.tile([D, D], bf16)
            nc.sync.dma_start(wx_sb, wx_d[:, :])
            w2c_sb = cpool.tile([D, 1], bf16)
            nc.sync.dma_start(w2c_sb, w2c_d[:, :])
            c2_sb = cpool.tile([128, 1], fp32)
            nc.sync.dma_start(c2_sb, c2_d[0, 0:1].broadcast_to((128, 1)))
            qpt_sb = cpool.tile([D, BSH], fp32)
            nc.sync.dma_start(qpt_sb, qpt_d[:, :])

            fout = fout_pool.tile([128, BSH], fp32)

            x16_tiles = [None] * NGRP
            wt_tiles = [None] * (NGRP // GPW)

            def do_final(ci):
                wt = wt_tiles[ci]
                for gg in range(GPW):
                    gsrc = ci * GPW + gg
                    xs = x16_tiles[gsrc]
                    for g in range(G):
                        bc = gsrc * G + g
                        bl = gg * G + g
                        for j in range(2):
                            nc.tensor.matmul(
                                fout[:, bc : bc + 1],
                                xs[0:P, g, j, :],
                                wt[:, j, bl : bl + 1],
                                start=(j == 0),
                                stop=(j == 1),
                            )
                    x16_tiles[gsrc] = None

            for gi in range(NGRP):
                b0 = gi * G
                ci = gi // GPW          # 32-b chunk index
                gl = gi % GPW           # group-in-chunk
                if gl == 0:
                    wps = wps_pool.tile([128, 2, WCHUNK], fp32, tag="wps")

                # ---- load fp32 pair tiles (1KB HBM chunks, 128 partitions)
                x32 = xpool.tile([PP, G, 2 * D], fp32, tag="x32")
                nc.sync.dma_start(
                    x32,
                    x_d[b0 : b0 + G, :, :].rearrange(
                        "b (p two) d -> p b (two d)", two=2
                    ),
                )
                # ---- cast to bf16 (DVE)
                x16 = x16pool.tile([PP, G, 2, D], bf16, tag="x16")
                nc.vector.tensor_copy(
                    x16.rearrange("p g two d -> p (g two d)"),
                    x32.rearrange("p g twod -> p (g twod)"),
                )
                x16_tiles[gi] = x16

                # ---- xT via bf16 PE transposes; int32-view PSUM drain
                xt = xtpool.tile([128, G, 2, CH], bf16, tag="xt")
                xtp = xtp_pool.tile([128, G, 2, CH], bf16, tag="xtp")
                for g in range(G):
                    for j in range(2):
                        nc.tensor.transpose(
                            xtp[:, g, j, 0:P],
                            x16[0:P, g, j, :],
                            ident16[0:P, 0:P],
                        )
                int32 = mybir.dt.int32
                nc.vector.tensor_copy(
                    xt.bitcast(int32).rearrange("p g two c -> p (g two c)"),
                    xtp.bitcast(int32).rearrange("p g two c -> p (g two c)"),
                )

                # ---- main matmul per (b, j): pre = Wx^T @ xT (N=100)
                h1 = h1pool.tile([128, G, 2, D], bf16, tag="h1")
                for g in range(G):
                    pre = pre_pool.tile([128, 2, 2 * CH], fp32, tag="pre")
                    for j in range(2):
                        nc.tensor.matmul(
                            pre[:, j, 0:P],
                            wx_sb,
                            xt[:, g, j, 0:P],
                            start=True,
                            stop=True,
                        )
                    bc = b0 + g
                    # ---- relu + Qp bias, PSUM -> SBUF bf16 (1 pass)
                    nc.scalar.activation(
                        h1[:, g, :, 0:P],
                        pre[:, :, 0:P],
                        AF.Relu,
                        bias=qpt_sb[:, bc : bc + 1],
                    )
                    # ---- logits: N=1 matmuls -> wT column pack
                    for j in range(2):
                        nc.tensor.matmul(
                            wps[:, j, gl * G + g : gl * G + g + 1],
                            h1[:, g, j, :],
                            w2c_sb,
                            start=True,
                            stop=True,
                        )

                # ---- sigmoid per 32-b chunk -> wT fp32; then finals
                if gl == GPW - 1:
                    wt = wtpool.tile([P, 2, WCHUNK], bf16, tag="wt")
                    nc.scalar.activation(
                        wt, wps[0:P], AF.Sigmoid, bias=c2_sb[0:P, 0:1]
                    )
                    wt_tiles[ci] = wt
                    do_final(ci)

            # ---- epilogue: transpose [d, b] -> [b, d] and store
            osb = finpool.tile([128, BSH], fp32)
            nc.scalar.activation(osb, fout, AF.Copy)
            obt = finpool.tile([128, BSH], fp32)
            for half in range(2):
                ot = pre_pool.tile([128, 2, 2 * CH], fp32, tag="pre")
                otv = ot.rearrange("p a c -> p (a c)")
                nc.tensor.transpose(
                    otv[:, 0:128], osb[:, half * 128 : half * 128 + 128], ident32
                )
                nc.scalar.activation(
                    obt[:, half * 128 : half * 128 + 128], otv[:, 0:128], AF.Copy
                )
                nc.sync.dma_start(
                    out_d[half * 128 : half * 128 + 128, :],
                    obt[:, half * 128 : half * 128 + 128],
                )
    nc.finalize()
    return nc


_NC_CACHE = {}


def _get_nc():
    if "nc" not in _NC_CACHE:
        _NC_CACHE["nc"] = _build_bass()
    return _NC_CACHE["nc"]


def _host_prep(inputs, query, W1, W2, bn1_gamma, bn1_beta, bn1_mean, bn1_var,
               bn2_gamma, bn2_beta, bn2_mean, bn2_var):
    x0 = np.asarray(inputs, np.float32)
    x = np.zeros((B, TPAD, D), np.float32)
    x[:, 0:T, :] = x0
    q = np.asarray(query, np.float64)
    W1 = np.asarray(W1, np.float64)
    W2 = np.asarray(W2, np.float64)
    s1 = np.asarray(bn1_gamma, np.float64) / np.sqrt(
        np.asarray(bn1_var, np.float64) + BN_EPS
    )
    W1s = s1[:, None] * W1
    Wx = W1s[0:D] + W1s[D : 2 * D]
    Wq = W1s[2 * D : 3 * D] - W1s[D : 2 * D]
    bias0 = (np.asarray(bn1_beta, np.float64) - np.asarray(bn1_mean, np.float64) * s1) @ W1
    Qp = q @ Wq + bias0                          # [B, D]
    s2 = np.asarray(bn2_gamma, np.float64) / np.sqrt(
        np.asarray(bn2_var, np.float64) + BN_EPS
    )
    W2p = s2 * W2[:, 0]                          # [D]
    c2 = float(
        (np.asarray(bn2_beta, np.float64) - np.asarray(bn2_mean, np.float64) * s2)
        @ W2[:, 0]
    )
    wx16 = np.ascontiguousarray(Wx.astype(BF16))
    w2c16 = np.ascontiguousarray(W2p.astype(BF16)[:, None])       # [D, 1]
    qpt = np.ascontiguousarray(Qp.astype(np.float32).T)           # [D, B]
    c2a = np.full((1, 1), c2, np.float32)
    return x, qpt, wx16, w2c16, c2a


def kernel(inputs, query, W1, W2,
           bn1_gamma, bn1_beta, bn1_mean, bn1_var,
           bn2_gamma, bn2_beta, bn2_mean, bn2_var):
    from concourse.bass_utils import run_bass_kernel_spmd

    x, qpt, wx16, w2c16, c2a = _host_prep(
        inputs, query, W1, W2, bn1_gamma, bn1_beta, bn1_mean, bn1_var,
        bn2_gamma, bn2_beta, bn2_mean, bn2_var)

    nc = _get_nc()
    in_maps = []
    for c in range(N_CORES):
        in_maps.append(
            {
                "x": x[c * BSH : (c + 1) * BSH],
                "qpt": np.ascontiguousarray(qpt[:, c * BSH : (c + 1) * BSH]),
                "wx": wx16,
                "w2c": w2c16,
                "c2": c2a,
            }
        )
    res = run_bass_kernel_spmd(nc, in_maps, core_ids=list(range(N_CORES)))
    out = np.concatenate([r["out"] for r in res.results], axis=0)
    return out.astype(np.float32)
